# revision 6
# baseline (speedup 1.0000x reference)
"""Trainium2 Bass kernel for 2-layer GAT + global mean pool + log_softmax.

Strategy (8 NeuronCores, dst-sharded graph parallel):
  - Nodes are padded to NV=50176 and split into 392 blocks of 128; core c owns
    blocks [c*49, (c+1)*49) (dst ownership).
  - Per GAT layer, a "table" of per-node rows [h(256) | a_src.h(4) | a_dst.h(4)]
    is computed by a matmul NEFF (phase M, node-sharded), assembled on host,
    and re-fed to all cores (the all-gather halo).
  - The edge NEFF (phase E) processes each core's edges grouped by dst block.
    Self-loop edges are excluded from the edge list and folded in as a local
    elementwise term. Per block, dma_gather fetches the source-node rows of
    all non-self edges (lo/hi table halves on SWDGE queues 0/1 so descriptor
    generation overlaps queue drain); the softmax weight
    ex = exp(leaky_relu(as_src + ad_dst)) is computed per edge; 0/1 indicator
    matrices ind (edge->dst) and indT (dst->edge), precomputed on host from
    the static edge schedule and streamed from DRAM, drive two matmul groups:
    indT @ ad expands the per-dst attention term to edges, and
    ind @ [ex*h | ex] accumulates [z | den] per dst node in PSUM.
    The softmax denominator divides out after aggregation; ELU + bias follow;
    layer-2 adds a pooling matmul with host-baked 1/count graph weights.
  - Final 64x10 classifier + log_softmax on host from per-core pooled partials.

Edge slots are padded per (block, table-half) to a uniform cross-core tile
schedule; pad slots gather row 0 and carry dst_local=255, which zeroes their
indicator columns so they contribute nothing.

dma_gather indices are int16, so the table is split at row 25088 (lo/hi) and
each block's edges are partitioned into lo/hi sub-gathers.
"""
import sys
import types
sys.path.insert(0, "/opt/trn_rl_repo")
import numpy as np
import ml_dtypes

# Install the NTFF profiling hook that the boot path skips when
# antenv.axon_hooks is absent (needed for exec_time_ns under trace=True).
if "antenv.axon_hooks" not in sys.modules:
    _m = types.ModuleType("antenv.axon_hooks")
    _m._hook = None
    _m.set_axon_ntff_profile_hook = lambda h: setattr(_m, "_hook", h)
    _m.get_axon_ntff_profile_hook = lambda: _m._hook
    sys.modules["antenv.axon_hooks"] = _m
    try:
        if "/root/.axon_site" not in sys.path:
            sys.path.insert(0, "/root/.axon_site")
        from trn_agent_boot.trn_boot import _ntff_profile_via_ctypes
        _hk = _ntff_profile_via_ctypes("/opt/axon/libaxon_pjrt.so")
        if _hk is not None:
            _m._hook = _hk
    except Exception:
        pass

import concourse.bacc as bacc
import concourse.bass as bass
import concourse.mybir as mybir
import concourse.tile as tile
from concourse import library_config
from concourse import bass_utils as _bu
from concourse.bass_utils import run_bass_kernel_spmd

_bu.upload_artifacts = lambda tmpdir: "local"

F32, BF16, I16 = mybir.dt.float32, mybir.dt.bfloat16, mybir.dt.int16
AF = mybir.ActivationFunctionType
OP = mybir.AluOpType

# problem constants (hardcoded per spec)
N, E = 50000, 800000
F_IN, HID, HEADS, NCLS, NGRAPH = 128, 64, 4, 10, 64
D = HID * HEADS            # 256
SLOPE = 0.2
NCORES = 8
BLK = 128
NB = 49                    # blocks per core
NODES_PC = NB * BLK        # 6272
NV = NCORES * NODES_PC     # 50176
SPLIT = NV // 2            # 25088
RB = 384                   # table row bf16 elems (768 B)
RC = 264                   # used row cols [h(256)|as(4)|ad(4)]

_CACHE = {}


# --------------------------------------------------------------------------
# host-side schedule (no self loops; those are handled locally on device)
# --------------------------------------------------------------------------
def build_schedule(src, dst):
    blk = dst // BLK
    order = np.argsort(blk, kind="stable")
    src_s, dst_s, blk_s = src[order], dst[order], blk[order]
    starts = np.searchsorted(blk_s, np.arange(392 + 1))
    per = []          # [core][b] -> (lo_src, hi_src, lo_dst, hi_dst)
    for c in range(NCORES):
        slots = []
        for b in range(NB):
            gb = c * NB + b
            s = src_s[starts[gb]:starts[gb + 1]]
            dl = dst_s[starts[gb]:starts[gb + 1]] - gb * BLK
            lo = s < SPLIT
            slots.append((s[lo], s[~lo] - SPLIT, dl[lo], dl[~lo]))
        per.append(slots)
    Tlo = np.zeros(NB, np.int64)
    Thi = np.zeros(NB, np.int64)
    for b in range(NB):
        for c in range(NCORES):
            lo, hi = per[c][b][0], per[c][b][1]
            Tlo[b] = max(Tlo[b], -(-len(lo) // BLK))
            Thi[b] = max(Thi[b], -(-len(hi) // BLK))
    return per, Tlo, Thi


def pack_idx(idx):
    """int16 index list (len % 128 == 0) -> [128, len//16] wrapped layout."""
    return np.tile(idx.reshape(-1, 16).T, (8, 1))


def host_arrays(per, Tlo, Thi):
    """Per-core DRAM input arrays for the edge NEFF.

    Returns per core: (idx_all [128, 8*TOT] i16, ind [128, TOT*128] bf16,
    indT [128, TOT*128] bf16)."""
    out = []
    ar128 = np.arange(128)
    for c in range(NCORES):
        idx_cols, ind_cols, indT_cols = [], [], []
        for b in range(NB):
            lo, hi, dlo, dhi = per[c][b]
            nlo, nhi = int(Tlo[b]) * BLK, int(Thi[b]) * BLK
            a = np.zeros(nlo, np.int64); a[:len(lo)] = lo
            bb = np.zeros(nhi, np.int64); bb[:len(hi)] = hi
            dd = np.full(nlo + nhi, 255, np.int64)
            dd[:len(dlo)] = dlo
            dd[nlo:nlo + len(dhi)] = dhi
            for arr, ntile in ((a, int(Tlo[b])), (bb, int(Thi[b]))):
                done = 0
                while done < ntile:
                    ck = min(8, ntile - done)
                    idx_cols.append(pack_idx(
                        arr[done * BLK:(done + ck) * BLK].astype(np.int16)))
                    done += ck
            D2 = dd.reshape(-1, BLK)                       # [T, 128] slot dsts
            eq = (D2[:, :, None] == ar128[None, None, :])  # [T, slot, d]
            ind_cols.append(eq.transpose(1, 0, 2).reshape(128, -1))
            indT_cols.append(eq.transpose(2, 0, 1).reshape(128, -1))
        idx_all = np.concatenate(idx_cols, axis=1)
        ind = np.concatenate(ind_cols, axis=1).astype(ml_dtypes.bfloat16)
        indT = np.concatenate(indT_cols, axis=1).astype(ml_dtypes.bfloat16)
        out.append((idx_all, ind, indT))
    return out


# --------------------------------------------------------------------------
# phase M NEFF: table shard = lhsT.T @ Wext  (K=256, zero-padded for layer 1)
# --------------------------------------------------------------------------
def build_phase_m():
    nc = bacc.Bacc("TRN2", target_bir_lowering=False, debug=False,
                   num_devices=NCORES)
    lhsT_in = nc.dram_tensor("lhsT", [2, 128, NODES_PC], F32, kind="ExternalInput")
    wext_in = nc.dram_tensor("wext", [2, 128, RC], F32, kind="ExternalInput")
    shard = nc.dram_tensor("shard", [NODES_PC, RC], F32, kind="ExternalOutput")
    with tile.TileContext(nc) as tc:
        with (
            tc.tile_pool(name="w", bufs=1) as wp,
            tc.tile_pool(name="x", bufs=1) as xp,
            tc.tile_pool(name="st", bufs=3) as stp,
            tc.tile_pool(name="ps", bufs=2, space="PSUM") as psp,
        ):
            w0 = wp.tile([128, RC], F32)
            w1 = wp.tile([128, RC], F32)
            nc.sync.dma_start(w0[:], wext_in[0])
            nc.sync.dma_start(w1[:], wext_in[1])
            xT0 = xp.tile([128, NODES_PC], F32)
            xT1 = xp.tile([128, NODES_PC], F32)
            nc.sync.dma_start(xT0[:], lhsT_in[0])
            nc.sync.dma_start(xT1[:], lhsT_in[1])
            for t in range(NB):
                ps = psp.tile([128, RC], F32, tag="ps")
                sl = bass.ts(t, 128)
                nc.tensor.matmul(ps[:], xT0[:, sl], w0[:], start=True, stop=False)
                nc.tensor.matmul(ps[:], xT1[:, sl], w1[:], start=False, stop=True)
                st = stp.tile([128, RC], F32, tag="st")
                nc.vector.tensor_copy(st[:], ps[:])
                nc.sync.dma_start(shard[sl, :], st[:])
    nc.compile()
    return nc


# --------------------------------------------------------------------------
# phase E NEFF: edge aggregation for one layer
# --------------------------------------------------------------------------
def build_phase_e(Tlo, Thi, TOT):
    T_MAX = int((Tlo + Thi).max())
    NIDX = 8 * TOT
    nc = bacc.Bacc("TRN2", target_bir_lowering=False, debug=False,
                   num_devices=NCORES, num_swdge_queues=2)
    t_lo = nc.dram_tensor("t_lo", [SPLIT, RB], BF16, kind="ExternalInput")
    t_hi = nc.dram_tensor("t_hi", [SPLIT, RB], BF16, kind="ExternalInput")
    idx_in = nc.dram_tensor("idx", [128, NIDX], I16, kind="ExternalInput")
    ind_in = nc.dram_tensor("ind", [128, TOT * BLK], BF16, kind="ExternalInput")
    indT_in = nc.dram_tensor("indT", [128, TOT * BLK], BF16, kind="ExternalInput")
    town_in = nc.dram_tensor("town", [NODES_PC, RC], F32, kind="ExternalInput")
    bias_in = nc.dram_tensor("bias", [128, D], F32, kind="ExternalInput")
    indg_in = nc.dram_tensor("indg", [NODES_PC, NGRAPH], F32, kind="ExternalInput")
    z_out = nc.dram_tensor("z_out", [NODES_PC, D], F32, kind="ExternalOutput")
    pool_out = nc.dram_tensor("pool_out", [NGRAPH, D], F32, kind="ExternalOutput")

    with tile.TileContext(nc) as tc:
        nc.gpsimd.load_library(library_config.mlp)
        with (
            tc.tile_pool(name="cst", bufs=1) as cst,
            tc.tile_pool(name="hg", bufs=3) as hgp,
            tc.tile_pool(name="hs", bufs=2) as hsp,
            tc.tile_pool(name="ix", bufs=3) as ixp,
            tc.tile_pool(name="sm", bufs=4) as smp,
            tc.tile_pool(name="zz", bufs=3) as zzp,
            tc.tile_pool(name="psad", bufs=2, space="PSUM") as psadp,
            tc.tile_pool(name="psz", bufs=2, space="PSUM") as pszp,
            tc.tile_pool(name="pspool", bufs=1, space="PSUM") as pspoolp,
        ):
            idx_all = cst.tile([128, NIDX], I16)
            nc.sync.dma_start(idx_all[:], idx_in[:])
            bias = cst.tile([128, D], F32)
            nc.sync.dma_start(bias[:], bias_in[:])
            ps_pool = pspoolp.tile([NGRAPH, D], F32)

            off = 0    # tile offset
            ioff = 0   # idx column offset
            for b in range(NB):
                tl, th = int(Tlo[b]), int(Thi[b])
                T = tl + th
                hg = hgp.tile([128, T_MAX, RB], BF16, tag="hg")
                # dma_gather tops out at 1024 indices (64 idx columns); chunk.
                # lo chunks on SWDGE queue 0, hi chunks on queue 1, issued
                # interleaved so one FIFO drains while the other fills.
                calls = []
                for q, (base, cnt, tab) in enumerate(
                        ((0, tl, t_lo), (tl, th, t_hi))):
                    done = 0
                    while done < cnt:
                        ck = min(8, cnt - done)
                        calls.append((q, base + done, ck, tab, ioff))
                        ioff += ck * 8
                        done += ck
                calls.sort(key=lambda cl: (-cl[2], cl[0]))
                for q, base, ck, tab, io in calls:
                    nc.gpsimd.dma_gather(
                        hg[:, base:base + ck, :], tab[:],
                        idx_all[:, io:io + ck * 8],
                        ck * BLK, ck * BLK, RB, queue_num=q)

                ind = ixp.tile([128, T_MAX * BLK], BF16, tag="ind")
                nc.sync.dma_start(ind[:, 0:T * BLK],
                                  ind_in[:, off * BLK:(off + T) * BLK])
                indT = ixp.tile([128, T_MAX * BLK], BF16, tag="indT")
                nc.sync.dma_start(indT[:, 0:T * BLK],
                                  indT_in[:, off * BLK:(off + T) * BLK])
                town = smp.tile([128, RC], F32, tag="town")
                nc.sync.dma_start(town[:], town_in[bass.ts(b, 128), :])
                adb = smp.tile([128, 4], BF16, tag="adb")
                nc.vector.tensor_copy(adb[:], town[:, 260:264])

                # adE[e] = ad[dst_e] via indT matmul per tile
                ps_ad = psadp.tile([128, 4 * T_MAX], F32, tag="psad")
                for t in range(T):
                    nc.tensor.matmul(ps_ad[:, t * 4:(t + 1) * 4],
                                     indT[:, t * BLK:(t + 1) * BLK],
                                     adb[:], start=True, stop=True)

                # ex = exp(leaky_relu(as + adE))   (lrelu+exp on Scalar engine)
                exbuf = smp.tile([128, T_MAX, 4], F32, tag="exbuf")
                nc.vector.tensor_tensor(
                    exbuf[:, 0:T, :], hg[:, 0:T, 256:260],
                    ps_ad[:, 0:4 * T].rearrange("p (t h) -> p t h", h=4), OP.add)
                flat = exbuf[:, 0:T, :].rearrange("p t h -> p (t h)")
                nc.vector.scalar_tensor_tensor(flat, flat, SLOPE, flat,
                                               OP.mult, OP.max)
                nc.scalar.activation(flat, flat, AF.Exp)

                # Hs = [ex * h | ex]  (bf16); ex tail written by Scalar engine
                hsall = hsp.tile([128, T_MAX, 260], BF16, tag="hsall")
                nc.vector.tensor_tensor(
                    hsall[:, 0:T, 0:256].rearrange("p t (h f) -> p t h f", h=4),
                    hg[:, 0:T, 0:256].rearrange("p t (h f) -> p t h f", h=4),
                    exbuf[:, 0:T, :].broadcast_to([128, T, 4, HID]), OP.mult)
                nc.scalar.activation(hsall[:, 0:T, 256:260],
                                     exbuf[:, 0:T, :], AF.Copy)

                # [z | den] accumulation over edge tiles
                ps_z = pszp.tile([128, 260], F32, tag="psz")
                for t in range(T):
                    nc.tensor.matmul(ps_z[:], ind[:, t * BLK:(t + 1) * BLK],
                                     hsall[:, t, :],
                                     start=(t == 0), stop=(t == T - 1))

                # self-loop: exS = exp(lrelu(as_own + ad_own)); den/z add
                exs = smp.tile([128, 4], F32, tag="exs")
                nc.vector.tensor_tensor(exs[:], town[:, 256:260],
                                        town[:, 260:264], OP.add)
                nc.vector.scalar_tensor_tensor(exs[:], exs[:], SLOPE, exs[:],
                                               OP.mult, OP.max)
                nc.scalar.activation(exs[:], exs[:], AF.Exp)
                selfc = zzp.tile([128, D], F32, tag="selfc")
                nc.vector.tensor_tensor(
                    selfc[:].rearrange("p (h f) -> p h f", h=4),
                    town[:, 0:256].rearrange("p (h f) -> p h f", h=4),
                    exs[:].broadcast_to([128, 4, HID]), OP.mult)

                # z = (agg + selfc) * rden + bias; elu
                den = smp.tile([128, 4], F32, tag="den")
                nc.vector.tensor_tensor(den[:], ps_z[:, 256:260], exs[:], OP.add)
                rden = smp.tile([128, 4], F32, tag="rden")
                nc.vector.reciprocal(rden[:], den[:])
                t0 = zzp.tile([128, D], F32, tag="t0")
                nc.vector.tensor_tensor(t0[:], ps_z[:, 0:256], selfc[:], OP.add)
                nc.vector.tensor_tensor(
                    t0[:].rearrange("p (h f) -> p h f", h=4),
                    t0[:].rearrange("p (h f) -> p h f", h=4),
                    rden[:].broadcast_to([128, 4, HID]), OP.mult)
                nc.vector.tensor_tensor(t0[:], t0[:], bias[:], OP.add)
                em = zzp.tile([128, D], F32, tag="em")
                nc.vector.tensor_scalar(em[:], t0[:], 0.0, None, OP.min)
                nc.scalar.activation(em[:], em[:], AF.Exp)
                zel = zzp.tile([128, D], F32, tag="zel")
                nc.vector.tensor_scalar(t0[:], t0[:], 0.0, None, OP.max)
                nc.vector.scalar_tensor_tensor(zel[:], em[:], -1.0, t0[:],
                                               OP.add, OP.add)
                nc.sync.dma_start(z_out[bass.ts(b, 128), :], zel[:])

                # pooling partial
                indg = smp.tile([128, NGRAPH], F32, tag="indg")
                nc.sync.dma_start(indg[:], indg_in[bass.ts(b, 128), :])
                nc.tensor.matmul(ps_pool[:], indg[:], zel[:],
                                 start=(b == 0), stop=(b == NB - 1))
                off += T

            poolsb = cst.tile([NGRAPH, D], F32)
            nc.vector.tensor_copy(poolsb[:], ps_pool[:])
            nc.sync.dma_start(pool_out[:], poolsb[:])
    nc.compile()
    return nc


# --------------------------------------------------------------------------
# kernel entry
# --------------------------------------------------------------------------
def kernel(x, edge_index, batch, W1, att_src1, att_dst1, b1,
           W2, att_src2, att_dst2, b2, lin_w, lin_b):
    x = np.asarray(x, np.float32)
    ei = np.asarray(edge_index, np.int64)
    batch = np.asarray(batch, np.int64)
    W1 = np.asarray(W1, np.float32); W2 = np.asarray(W2, np.float32)
    a_s1 = np.asarray(att_src1, np.float32); a_d1 = np.asarray(att_dst1, np.float32)
    a_s2 = np.asarray(att_src2, np.float32); a_d2 = np.asarray(att_dst2, np.float32)
    b1 = np.asarray(b1, np.float32); b2 = np.asarray(b2, np.float32)
    lin_w = np.asarray(lin_w, np.float32); lin_b = np.asarray(lin_b, np.float32)

    # self loops excluded from the edge schedule (folded in on device);
    # drop any explicit self edges from the input to avoid double counting
    nonself = ei[0] != ei[1]
    src, dst = ei[0][nonself], ei[1][nonself]

    per, Tlo, Thi = build_schedule(src, dst)
    TOT = int((Tlo + Thi).sum())
    arrays = host_arrays(per, Tlo, Thi)

    if "m" not in _CACHE:
        _CACHE["m"] = build_phase_m()
    key = ("e", tuple(Tlo), tuple(Thi))
    if key not in _CACHE:
        _CACHE[key] = build_phase_e(Tlo, Thi, TOT)
    nc_m, nc_e = _CACHE["m"], _CACHE[key]

    def amat(a_src, a_dst):
        m = np.zeros((D, 8), np.float32)
        for hd in range(HEADS):
            m[hd * HID:(hd + 1) * HID, hd] = a_src[hd]
            m[hd * HID:(hd + 1) * HID, 4 + hd] = a_dst[hd]
        return m

    def wext(W, a_src, a_dst):
        Fin = W.shape[0]
        we = np.zeros((2, 128, RC), np.float32)
        full = np.concatenate([W, W @ amat(a_src, a_dst)], axis=1)  # [Fin, 264]
        we.reshape(256, RC)[:Fin] = full
        return we

    cnt = np.bincount(batch, minlength=NGRAPH).astype(np.float32)
    pw = np.zeros((NV, NGRAPH), np.float32)
    pw[np.arange(N), batch] = (1.0 / np.maximum(cnt, 1.0))[batch]
    zeros_pw = np.zeros((NODES_PC, NGRAPH), np.float32)

    exec_ns = 0.0

    import os
    want_trace = os.environ.get("BASS_GAT_TRACE", "0") == "1"

    def run(nc, maps):
        nonlocal exec_ns
        if want_trace:
            try:
                res = run_bass_kernel_spmd(nc, maps,
                                           core_ids=list(range(NCORES)),
                                           trace=True)
                if res.exec_time_ns:
                    exec_ns += res.exec_time_ns
                    print(f"kernel: run exec_time = {res.exec_time_ns:.0f} ns")
                return res.results
            except Exception as exc:
                print(f"kernel: traced run failed ({exc!r}); rerunning untraced")
        res = run_bass_kernel_spmd(nc, maps, core_ids=list(range(NCORES)),
                                   trace=False)
        return res.results

    def phase_m(lhsT_full, we):
        maps = []
        for c in range(NCORES):
            lt = lhsT_full[:, :, c * NODES_PC:(c + 1) * NODES_PC]
            maps.append({"lhsT": lt, "wext": we})
        return run(nc_m, maps)

    def phase_e(shards, bvec, pool_w):
        table = np.concatenate([s["shard"] for s in shards], axis=0)  # [NV,264]
        tbl = np.zeros((NV, RB), ml_dtypes.bfloat16)
        tbl[:, :RC] = table.astype(ml_dtypes.bfloat16)
        t_lo, t_hi = tbl[:SPLIT], tbl[SPLIT:]
        bias_bc = np.tile(bvec, (128, 1)).astype(np.float32)
        maps = []
        for c in range(NCORES):
            idx_all, ind, indT = arrays[c]
            sl = slice(c * NODES_PC, (c + 1) * NODES_PC)
            maps.append({
                "t_lo": t_lo, "t_hi": t_hi, "idx": idx_all,
                "ind": ind, "indT": indT,
                "town": shards[c]["shard"],
                "bias": bias_bc,
                "indg": np.ascontiguousarray(pool_w[sl]) if pool_w is not None
                        else zeros_pw,
            })
        res = run(nc_e, maps)
        return res, table

    # ---- layer 1
    xT_full = np.zeros((2, 128, NV), np.float32)
    xT_full.reshape(256, NV)[:F_IN, :N] = x.T
    shards = phase_m(xT_full, wext(W1, a_s1, a_d1))

    res1, _ = phase_e(shards, b1, None)
    z1 = np.concatenate([r["z_out"] for r in res1], axis=0)        # [NV, 256]

    # ---- layer 2
    z1T_full = np.ascontiguousarray(z1.T).reshape(2, 128, NV)
    shards2 = phase_m(z1T_full, wext(W2, a_s2, a_d2))

    res2, _ = phase_e(shards2, b2, pw)
    pooled = np.sum([r["pool_out"] for r in res2], axis=0)         # [64, 256]

    # ---- classifier + log_softmax (host)
    logits = pooled @ lin_w + lin_b
    logits -= logits.max(axis=1, keepdims=True)
    out = logits - np.log(np.exp(logits).sum(axis=1, keepdims=True))

    kernel.last_exec_ns = exec_ns
    return out.astype(np.float32)


kernel.last_exec_ns = None


# revision 12
# speedup vs baseline: 1.1736x; 1.1736x over previous
"""Trainium2 Bass kernel for 2-layer GAT + global mean pool + log_softmax.

Strategy (8 NeuronCores, dst-sharded graph parallel):
  - Nodes are padded to NV=50176 and split into 392 blocks of 128; core c owns
    blocks [c*49, (c+1)*49) (dst ownership).
  - Per GAT layer, a "table" of per-node rows [h(256) | a_src.h(4) | a_dst.h(4)]
    is computed by a matmul NEFF (phase M, node-sharded), assembled on host,
    and re-fed to all cores (the all-gather halo).
  - The edge NEFF (phase E) processes each core's edges grouped by dst block.
    Self-loop edges are excluded from the edge list and folded in as a local
    elementwise term. Per block, dma_gather fetches the source-node rows of
    all non-self edges (lo/hi table halves on SWDGE queues 0/1 so descriptor
    generation overlaps queue drain); the softmax weight
    ex = exp(leaky_relu(as_src + ad_dst)) is computed per edge; 0/1 indicator
    matrices ind (edge->dst) and indT (dst->edge), precomputed on host from
    the static edge schedule and streamed from DRAM, drive two matmul groups:
    indT @ ad expands the per-dst attention term to edges, and
    ind @ [ex*h | ex] accumulates [z | den] per dst node in PSUM.
    The softmax denominator divides out after aggregation; ELU + bias follow;
    layer-2 adds a pooling matmul with host-baked 1/count graph weights.
  - Final 64x10 classifier + log_softmax on host from per-core pooled partials.

Edge slots are padded per (block, table-half) to a uniform cross-core tile
schedule; pad slots gather row 0 and carry dst_local=255, which zeroes their
indicator columns so they contribute nothing.

dma_gather indices are int16, so the table is split at row 25088 (lo/hi) and
each block's edges are partitioned into lo/hi sub-gathers.
"""
import sys
import types
sys.path.insert(0, "/opt/trn_rl_repo")
import numpy as np
import ml_dtypes

# Install the NTFF profiling hook that the boot path skips when
# antenv.axon_hooks is absent (needed for exec_time_ns under trace=True).
if "antenv.axon_hooks" not in sys.modules:
    _m = types.ModuleType("antenv.axon_hooks")
    _m._hook = None
    _m.set_axon_ntff_profile_hook = lambda h: setattr(_m, "_hook", h)
    _m.get_axon_ntff_profile_hook = lambda: _m._hook
    sys.modules["antenv.axon_hooks"] = _m
    try:
        if "/root/.axon_site" not in sys.path:
            sys.path.insert(0, "/root/.axon_site")
        from trn_agent_boot.trn_boot import _ntff_profile_via_ctypes
        _hk = _ntff_profile_via_ctypes("/opt/axon/libaxon_pjrt.so")
        if _hk is not None:
            _m._hook = _hk
    except Exception:
        pass

import concourse.bacc as bacc
import concourse.bass as bass
import concourse.mybir as mybir
import concourse.tile as tile
from concourse import library_config
from concourse import bass_utils as _bu
from concourse.bass_utils import run_bass_kernel_spmd

_bu.upload_artifacts = lambda tmpdir: "local"

F32, BF16, I16 = mybir.dt.float32, mybir.dt.bfloat16, mybir.dt.int16
AF = mybir.ActivationFunctionType
OP = mybir.AluOpType

# problem constants (hardcoded per spec)
N, E = 50000, 800000
F_IN, HID, HEADS, NCLS, NGRAPH = 128, 64, 4, 10, 64
D = HID * HEADS            # 256
SLOPE = 0.2
NCORES = 8
BLK = 128
NB = 49                    # blocks per core
NODES_PC = NB * BLK        # 6272
NV = NCORES * NODES_PC     # 50176
SPLIT = NV // 2            # 25088
RB = 384                   # table row bf16 elems (768 B)
RC = 264                   # used row cols [h(256)|as(4)|ad(4)]

_CACHE = {}


# --------------------------------------------------------------------------
# host-side schedule (no self loops; those are handled locally on device)
# --------------------------------------------------------------------------
def build_schedule(src, dst):
    blk = dst // BLK
    order = np.argsort(blk, kind="stable")
    src_s, dst_s, blk_s = src[order], dst[order], blk[order]
    starts = np.searchsorted(blk_s, np.arange(392 + 1))
    per = []          # [core][b] -> (lo_src, hi_src, lo_dst, hi_dst)
    for c in range(NCORES):
        slots = []
        for b in range(NB):
            gb = c * NB + b
            s = src_s[starts[gb]:starts[gb + 1]]
            dl = dst_s[starts[gb]:starts[gb + 1]] - gb * BLK
            lo = s < SPLIT
            slots.append((s[lo], s[~lo] - SPLIT, dl[lo], dl[~lo]))
        per.append(slots)
    Tlo = np.zeros(NB, np.int64)
    Thi = np.zeros(NB, np.int64)
    for b in range(NB):
        for c in range(NCORES):
            lo, hi = per[c][b][0], per[c][b][1]
            Tlo[b] = max(Tlo[b], -(-len(lo) // BLK))
            Thi[b] = max(Thi[b], -(-len(hi) // BLK))
    return per, Tlo, Thi


def pack_idx(idx):
    """int16 index list (len % 128 == 0) -> [128, len//16] wrapped layout."""
    return np.tile(idx.reshape(-1, 16).T, (8, 1))


def host_arrays(per, Tlo, Thi):
    """Per-core DRAM input arrays for the edge NEFF.

    Returns per core: (idx_all [128, 8*TOT] i16, indT [128, TOT*128] bf16,
    dstloc [128, TOT] bf16)."""
    out = []
    ar128 = np.arange(128)
    for c in range(NCORES):
        idx_cols, indT_cols, dl_cols = [], [], []
        for b in range(NB):
            lo, hi, dlo, dhi = per[c][b]
            nlo, nhi = int(Tlo[b]) * BLK, int(Thi[b]) * BLK
            a = np.zeros(nlo, np.int64); a[:len(lo)] = lo
            bb = np.zeros(nhi, np.int64); bb[:len(hi)] = hi
            dd = np.full(nlo + nhi, 255, np.int64)
            dd[:len(dlo)] = dlo
            dd[nlo:nlo + len(dhi)] = dhi
            for arr, ntile in ((a, int(Tlo[b])), (bb, int(Thi[b]))):
                done = 0
                while done < ntile:
                    ck = min(8, ntile - done)
                    idx_cols.append(pack_idx(
                        arr[done * BLK:(done + ck) * BLK].astype(np.int16)))
                    done += ck
            D2 = dd.reshape(-1, BLK)                       # [T, 128] slot dsts
            eq = (D2[:, :, None] == ar128[None, None, :])  # [T, slot, d]
            indT_cols.append(eq.transpose(2, 0, 1).reshape(128, -1))
            dl_cols.append(D2.T)                           # [128, T]
        idx_all = np.concatenate(idx_cols, axis=1)
        indT = np.concatenate(indT_cols, axis=1).astype(ml_dtypes.bfloat16)
        dstloc = np.concatenate(dl_cols, axis=1).astype(ml_dtypes.bfloat16)
        out.append((idx_all, indT, dstloc))
    return out


# --------------------------------------------------------------------------
# phase M NEFF: table shard = lhsT.T @ Wext  (K=256, zero-padded for layer 1)
# --------------------------------------------------------------------------
def build_phase_m():
    nc = bacc.Bacc("TRN2", target_bir_lowering=False, debug=False,
                   num_devices=NCORES)
    lhsT_in = nc.dram_tensor("lhsT", [2, 128, NODES_PC], F32, kind="ExternalInput")
    wext_in = nc.dram_tensor("wext", [2, 128, RC], F32, kind="ExternalInput")
    shard = nc.dram_tensor("shard", [NODES_PC, RC], F32, kind="ExternalOutput")
    with tile.TileContext(nc) as tc:
        with (
            tc.tile_pool(name="w", bufs=1) as wp,
            tc.tile_pool(name="x", bufs=1) as xp,
            tc.tile_pool(name="st", bufs=3) as stp,
            tc.tile_pool(name="ps", bufs=2, space="PSUM") as psp,
        ):
            w0 = wp.tile([128, RC], F32)
            w1 = wp.tile([128, RC], F32)
            nc.sync.dma_start(w0[:], wext_in[0])
            nc.sync.dma_start(w1[:], wext_in[1])
            xT0 = xp.tile([128, NODES_PC], F32)
            xT1 = xp.tile([128, NODES_PC], F32)
            nc.sync.dma_start(xT0[:], lhsT_in[0])
            nc.sync.dma_start(xT1[:], lhsT_in[1])
            for t in range(NB):
                ps = psp.tile([128, RC], F32, tag="ps")
                sl = bass.ts(t, 128)
                nc.tensor.matmul(ps[:], xT0[:, sl], w0[:], start=True, stop=False)
                nc.tensor.matmul(ps[:], xT1[:, sl], w1[:], start=False, stop=True)
                st = stp.tile([128, RC], F32, tag="st")
                nc.vector.tensor_copy(st[:], ps[:])
                nc.sync.dma_start(shard[sl, :], st[:])
    nc.compile()
    return nc


# --------------------------------------------------------------------------
# phase E NEFF: edge aggregation for one layer
# --------------------------------------------------------------------------
def build_phase_e(Tlo, Thi, TOT):
    T_MAX = int((Tlo + Thi).max())
    NIDX = 8 * TOT
    nc = bacc.Bacc("TRN2", target_bir_lowering=False, debug=False,
                   num_devices=NCORES, num_swdge_queues=4)
    t_lo = nc.dram_tensor("t_lo", [SPLIT, RB], BF16, kind="ExternalInput")
    t_hi = nc.dram_tensor("t_hi", [SPLIT, RB], BF16, kind="ExternalInput")
    idx_in = nc.dram_tensor("idx", [128, NIDX], I16, kind="ExternalInput")
    indT_in = nc.dram_tensor("indT", [128, TOT * BLK], BF16, kind="ExternalInput")
    dstloc_in = nc.dram_tensor("dstloc", [128, TOT], BF16, kind="ExternalInput")
    iota_in = nc.dram_tensor("iota", [128, T_MAX * BLK], BF16,
                             kind="ExternalInput")
    town_in = nc.dram_tensor("town", [NODES_PC, RC], F32, kind="ExternalInput")
    ado_in = nc.dram_tensor("ado", [NODES_PC, 4], BF16, kind="ExternalInput")
    bias_in = nc.dram_tensor("bias", [128, D], F32, kind="ExternalInput")
    indg_in = nc.dram_tensor("indg", [NODES_PC, NGRAPH], F32, kind="ExternalInput")
    z_out = nc.dram_tensor("z_out", [NODES_PC, D], F32, kind="ExternalOutput")
    pool_out = nc.dram_tensor("pool_out", [NGRAPH, D], F32, kind="ExternalOutput")

    with tile.TileContext(nc) as tc:
        nc.gpsimd.load_library(library_config.mlp)
        with (
            tc.tile_pool(name="cst", bufs=1) as cst,
            tc.tile_pool(name="hg", bufs=3) as hgp,
            tc.tile_pool(name="hs", bufs=2) as hsp,
            tc.tile_pool(name="ix", bufs=3) as ixp,
            tc.tile_pool(name="sm", bufs=4) as smp,
            tc.tile_pool(name="zz", bufs=3) as zzp,
            tc.tile_pool(name="psad", bufs=2, space="PSUM") as psadp,
            tc.tile_pool(name="psz", bufs=2, space="PSUM") as pszp,
            tc.tile_pool(name="pspool", bufs=1, space="PSUM") as pspoolp,
        ):
            idx_all = cst.tile([128, NIDX], I16)
            nc.sync.dma_start(idx_all[:], idx_in[:])
            bias = cst.tile([128, D], F32)
            nc.sync.dma_start(bias[:], bias_in[:])
            dstloc = cst.tile([128, TOT], BF16)
            nc.sync.dma_start(dstloc[:], dstloc_in[:])
            iota = cst.tile([128, T_MAX * BLK], BF16)
            nc.sync.dma_start(iota[:], iota_in[:])
            ps_pool = pspoolp.tile([NGRAPH, D], F32)

            off = 0    # tile offset
            ioff = 0   # idx column offset
            for b in range(NB):
                tl, th = int(Tlo[b]), int(Thi[b])
                T = tl + th
                hg = hgp.tile([128, T_MAX, RB], BF16, tag="hg")
                # dma_gather tops out at 1024 indices (64 idx columns); chunk.
                # lo chunks on SWDGE queue 0, hi chunks on queue 1, issued
                # interleaved so one FIFO drains while the other fills.
                calls = []
                for h, (base, cnt, tab) in enumerate(
                        ((0, tl, t_lo), (tl, th, t_hi))):
                    done = 0
                    ci = 0
                    while done < cnt:
                        ck = min(8, cnt - done)
                        calls.append((h * 2 + ci % 2, base + done, ck, tab,
                                      ioff))
                        ioff += ck * 8
                        done += ck
                        ci += 1
                calls.sort(key=lambda cl: (-cl[2], cl[0]))
                for q, base, ck, tab, io in calls:
                    nc.gpsimd.dma_gather(
                        hg[:, base:base + ck, :], tab[:],
                        idx_all[:, io:io + ck * 8],
                        ck * BLK, ck * BLK, RB, queue_num=q,
                        single_packet=False)

                # ind[e, d] = (dst_local[e] == d), built on DVE from a
                # contiguous iota constant (cheaper than streaming from DRAM)
                ind = ixp.tile([128, T_MAX * BLK], BF16, tag="ind")
                nc.vector.tensor_tensor(
                    ind[:, 0:T * BLK].rearrange("p (t f) -> p t f", f=BLK),
                    iota[:, 0:T * BLK].rearrange("p (t f) -> p t f", f=BLK),
                    dstloc[:, off:off + T].broadcast_to([128, T, BLK]),
                    OP.is_equal)
                indT = ixp.tile([128, T_MAX * BLK], BF16, tag="indT")
                nc.sync.dma_start(indT[:, 0:T * BLK],
                                  indT_in[:, off * BLK:(off + T) * BLK])
                town = smp.tile([128, RC], F32, tag="town")
                nc.sync.dma_start(town[:], town_in[bass.ts(b, 128), :])
                adb = smp.tile([128, 4], BF16, tag="adb")
                nc.sync.dma_start(adb[:], ado_in[bass.ts(b, 128), :])

                # adE[e] = ad[dst_e] via indT matmul per tile
                ps_ad = psadp.tile([128, 4 * T_MAX], F32, tag="psad")
                for t in range(T):
                    nc.tensor.matmul(ps_ad[:, t * 4:(t + 1) * 4],
                                     indT[:, t * BLK:(t + 1) * BLK],
                                     adb[:], start=True, stop=True)

                # ex = exp(leaky_relu(as + adE))   (lrelu+exp on Scalar engine)
                exbuf = smp.tile([128, T_MAX, 4], F32, tag="exbuf")
                nc.vector.tensor_tensor(
                    exbuf[:, 0:T, :], hg[:, 0:T, 256:260],
                    ps_ad[:, 0:4 * T].rearrange("p (t h) -> p t h", h=4), OP.add)
                flat = exbuf[:, 0:T, :].rearrange("p t h -> p (t h)")
                nc.vector.scalar_tensor_tensor(flat, flat, SLOPE, flat,
                                               OP.mult, OP.max)
                nc.scalar.activation(flat, flat, AF.Exp)

                # Hs = [ex * h | ex]  (bf16); ex tail written by Scalar engine
                hsall = hsp.tile([128, T_MAX, 260], BF16, tag="hsall")
                nc.vector.tensor_tensor(
                    hsall[:, 0:T, 0:256].rearrange("p t (h f) -> p t h f", h=4),
                    hg[:, 0:T, 0:256].rearrange("p t (h f) -> p t h f", h=4),
                    exbuf[:, 0:T, :].broadcast_to([128, T, 4, HID]), OP.mult)
                nc.scalar.activation(hsall[:, 0:T, 256:260],
                                     exbuf[:, 0:T, :], AF.Copy)

                # [z | den] accumulation over edge tiles
                ps_z = pszp.tile([128, 260], F32, tag="psz")
                for t in range(T):
                    nc.tensor.matmul(ps_z[:], ind[:, t * BLK:(t + 1) * BLK],
                                     hsall[:, t, :],
                                     start=(t == 0), stop=(t == T - 1))

                # self-loop: exS = exp(lrelu(as_own + ad_own)); den/z add
                exs = smp.tile([128, 4], F32, tag="exs")
                nc.vector.tensor_tensor(exs[:], town[:, 256:260],
                                        town[:, 260:264], OP.add)
                nc.vector.scalar_tensor_tensor(exs[:], exs[:], SLOPE, exs[:],
                                               OP.mult, OP.max)
                nc.scalar.activation(exs[:], exs[:], AF.Exp)
                selfc = zzp.tile([128, D], F32, tag="selfc")
                nc.vector.tensor_tensor(
                    selfc[:].rearrange("p (h f) -> p h f", h=4),
                    town[:, 0:256].rearrange("p (h f) -> p h f", h=4),
                    exs[:].broadcast_to([128, 4, HID]), OP.mult)

                # z = (agg + selfc) * rden + bias; elu
                den = smp.tile([128, 4], F32, tag="den")
                nc.vector.tensor_tensor(den[:], ps_z[:, 256:260], exs[:], OP.add)
                rden = smp.tile([128, 4], F32, tag="rden")
                nc.vector.reciprocal(rden[:], den[:])
                t0 = zzp.tile([128, D], F32, tag="t0")
                nc.vector.tensor_tensor(t0[:], ps_z[:, 0:256], selfc[:], OP.add)
                nc.vector.tensor_tensor(
                    t0[:].rearrange("p (h f) -> p h f", h=4),
                    t0[:].rearrange("p (h f) -> p h f", h=4),
                    rden[:].broadcast_to([128, 4, HID]), OP.mult)
                nc.vector.tensor_tensor(t0[:], t0[:], bias[:], OP.add)
                em = zzp.tile([128, D], F32, tag="em")
                nc.vector.tensor_scalar(em[:], t0[:], 0.0, None, OP.min)
                nc.scalar.activation(em[:], em[:], AF.Exp)
                zel = zzp.tile([128, D], F32, tag="zel")
                nc.vector.tensor_scalar(t0[:], t0[:], 0.0, None, OP.max)
                nc.vector.scalar_tensor_tensor(zel[:], em[:], -1.0, t0[:],
                                               OP.add, OP.add)
                nc.sync.dma_start(z_out[bass.ts(b, 128), :], zel[:])

                # pooling partial
                indg = smp.tile([128, NGRAPH], F32, tag="indg")
                nc.sync.dma_start(indg[:], indg_in[bass.ts(b, 128), :])
                nc.tensor.matmul(ps_pool[:], indg[:], zel[:],
                                 start=(b == 0), stop=(b == NB - 1))
                off += T

            poolsb = cst.tile([NGRAPH, D], F32)
            nc.vector.tensor_copy(poolsb[:], ps_pool[:])
            nc.sync.dma_start(pool_out[:], poolsb[:])
    nc.compile()
    return nc


# --------------------------------------------------------------------------
# kernel entry
# --------------------------------------------------------------------------
def kernel(x, edge_index, batch, W1, att_src1, att_dst1, b1,
           W2, att_src2, att_dst2, b2, lin_w, lin_b):
    x = np.asarray(x, np.float32)
    ei = np.asarray(edge_index, np.int64)
    batch = np.asarray(batch, np.int64)
    W1 = np.asarray(W1, np.float32); W2 = np.asarray(W2, np.float32)
    a_s1 = np.asarray(att_src1, np.float32); a_d1 = np.asarray(att_dst1, np.float32)
    a_s2 = np.asarray(att_src2, np.float32); a_d2 = np.asarray(att_dst2, np.float32)
    b1 = np.asarray(b1, np.float32); b2 = np.asarray(b2, np.float32)
    lin_w = np.asarray(lin_w, np.float32); lin_b = np.asarray(lin_b, np.float32)

    # self loops excluded from the edge schedule (folded in on device);
    # drop any explicit self edges from the input to avoid double counting
    nonself = ei[0] != ei[1]
    src, dst = ei[0][nonself], ei[1][nonself]

    per, Tlo, Thi = build_schedule(src, dst)
    TOT = int((Tlo + Thi).sum())
    arrays = host_arrays(per, Tlo, Thi)

    if "m" not in _CACHE:
        _CACHE["m"] = build_phase_m()
    key = ("e", tuple(Tlo), tuple(Thi))
    if key not in _CACHE:
        _CACHE[key] = build_phase_e(Tlo, Thi, TOT)
    nc_m, nc_e = _CACHE["m"], _CACHE[key]

    def amat(a_src, a_dst):
        m = np.zeros((D, 8), np.float32)
        for hd in range(HEADS):
            m[hd * HID:(hd + 1) * HID, hd] = a_src[hd]
            m[hd * HID:(hd + 1) * HID, 4 + hd] = a_dst[hd]
        return m

    def wext(W, a_src, a_dst):
        Fin = W.shape[0]
        we = np.zeros((2, 128, RC), np.float32)
        full = np.concatenate([W, W @ amat(a_src, a_dst)], axis=1)  # [Fin, 264]
        we.reshape(256, RC)[:Fin] = full
        return we

    cnt = np.bincount(batch, minlength=NGRAPH).astype(np.float32)
    pw = np.zeros((NV, NGRAPH), np.float32)
    pw[np.arange(N), batch] = (1.0 / np.maximum(cnt, 1.0))[batch]
    zeros_pw = np.zeros((NODES_PC, NGRAPH), np.float32)

    exec_ns = 0.0

    import os
    want_trace = os.environ.get("BASS_GAT_TRACE", "0") == "1"

    def run(nc, maps):
        nonlocal exec_ns
        if want_trace:
            try:
                res = run_bass_kernel_spmd(nc, maps,
                                           core_ids=list(range(NCORES)),
                                           trace=True)
                if res.exec_time_ns:
                    exec_ns += res.exec_time_ns
                    print(f"kernel: run exec_time = {res.exec_time_ns:.0f} ns")
                return res.results
            except Exception as exc:
                print(f"kernel: traced run failed ({exc!r}); rerunning untraced")
        res = run_bass_kernel_spmd(nc, maps, core_ids=list(range(NCORES)),
                                   trace=False)
        return res.results

    def phase_m(lhsT_full, we):
        maps = []
        for c in range(NCORES):
            lt = lhsT_full[:, :, c * NODES_PC:(c + 1) * NODES_PC]
            maps.append({"lhsT": lt, "wext": we})
        return run(nc_m, maps)

    T_MAX = int((Tlo + Thi).max())
    iota_np = np.tile(np.arange(BLK), (128, T_MAX)).astype(ml_dtypes.bfloat16)

    def phase_e(shards, bvec, pool_w):
        table = np.concatenate([s["shard"] for s in shards], axis=0)  # [NV,264]
        tbl = np.zeros((NV, RB), ml_dtypes.bfloat16)
        tbl[:, :RC] = table.astype(ml_dtypes.bfloat16)
        t_lo, t_hi = tbl[:SPLIT], tbl[SPLIT:]
        bias_bc = np.tile(bvec, (128, 1)).astype(np.float32)
        maps = []
        for c in range(NCORES):
            idx_all, indT, dstloc = arrays[c]
            sl = slice(c * NODES_PC, (c + 1) * NODES_PC)
            maps.append({
                "t_lo": t_lo, "t_hi": t_hi, "idx": idx_all,
                "indT": indT, "dstloc": dstloc, "iota": iota_np,
                "town": shards[c]["shard"],
                "ado": np.ascontiguousarray(
                    shards[c]["shard"][:, 260:264]).astype(ml_dtypes.bfloat16),
                "bias": bias_bc,
                "indg": np.ascontiguousarray(pool_w[sl]) if pool_w is not None
                        else zeros_pw,
            })
        res = run(nc_e, maps)
        return res, table

    # ---- layer 1
    xT_full = np.zeros((2, 128, NV), np.float32)
    xT_full.reshape(256, NV)[:F_IN, :N] = x.T
    shards = phase_m(xT_full, wext(W1, a_s1, a_d1))

    res1, _ = phase_e(shards, b1, None)
    z1 = np.concatenate([r["z_out"] for r in res1], axis=0)        # [NV, 256]

    # ---- layer 2
    z1T_full = np.ascontiguousarray(z1.T).reshape(2, 128, NV)
    shards2 = phase_m(z1T_full, wext(W2, a_s2, a_d2))

    res2, _ = phase_e(shards2, b2, pw)
    pooled = np.sum([r["pool_out"] for r in res2], axis=0)         # [64, 256]

    # ---- classifier + log_softmax (host)
    logits = pooled @ lin_w + lin_b
    logits -= logits.max(axis=1, keepdims=True)
    out = logits - np.log(np.exp(logits).sum(axis=1, keepdims=True))

    kernel.last_exec_ns = exec_ns
    return out.astype(np.float32)


kernel.last_exec_ns = None
